# revision 1
# baseline (speedup 1.0000x reference)
"""Leaky-integrator linear recurrence kernel for Trainium2.

u_t = TAU * u_{t-1} + x_t along the last (time) axis of x[32, 1024, 2048] f32.

Strategy: data-parallel across 8 NeuronCores (4 batches each). Per core the
shard is viewed as [4096 rows, 2048 time]; rows map to SBUF partitions
(32 tiles of [128, 2048]) and the recurrence runs along the free dimension
via the Vector engine's hardware scan instruction (TensorTensorScanArith):
state = data0*state + data1 with data0 = TAU.

The walrus build in this container allows at most ONE embedded sync-wait
per engine instruction (two on EventSemaphore); Tile's wait assignment can
attach several. _split_excess_waits() hoists the extras onto standalone
EventSemaphore instructions inserted immediately before, on the same
engine — conservative (the engine waits a bit earlier than strictly
needed) but correct, since every awaited semaphore's producer precedes the
waiter in the scheduled program order.
"""

import numpy as np

import concourse.bass as bass
import concourse.mybir as mybir
from concourse.bass_utils import run_bass_kernel_spmd
from concourse.tile import TileContext

TAU = 0.9
B, F, T = 32, 1024, 2048
N_CORES = 8
B_PER_CORE = B // N_CORES          # 4
ROWS = B_PER_CORE * F              # 4096 independent recurrences per core
P = 128
N_TILES = ROWS // P                # 32

_nc_cache = None
last_results = None  # BassKernelResults from the most recent run (for test.py)


def _split_excess_waits(nc: bass.Bass) -> None:
    for fn in nc.m.functions:
        for blk in fn.blocks:
            out = []
            changed = False
            for inst in blk.instructions:
                si = inst.sync_info
                waits = list(si.on_wait) if si is not None else []
                cap = 2 if inst.opcode == "EventSemaphore" else 1
                if len(waits) <= cap:
                    out.append(inst)
                    continue
                changed = True
                # On DMAs keep a queue-ordering (DMAHW*) wait embedded so
                # queue-level throttling stays at the queue; otherwise keep
                # the last wait.
                keep_idx = len(waits) - 1
                if inst.opcode == "DMACopy":
                    for k, w in enumerate(waits):
                        if (w.ant_name or "").startswith("DMA"):
                            keep_idx = k
                            break
                rest = [w for j, w in enumerate(waits) if j != keep_idx]
                for j in range(0, len(rest), 2):
                    out.append(
                        mybir.InstEventSemaphore(
                            name=f"{inst.name}-xw{j}",
                            opcode="EventSemaphore",
                            engine=inst.engine,
                            debug=inst.debug,
                            sync_info=mybir.SyncInfo(
                                on_wait=rest[j : j + 2], on_update=[]
                            ),
                        )
                    )
                inst.sync_info = mybir.SyncInfo(
                    on_wait=[waits[keep_idx]], on_update=list(si.on_update)
                )
                out.append(inst)
            if changed:
                blk.instructions = out


K_SUP = 2                          # 128-row groups per super-tile
N_SUP = N_TILES // K_SUP           # 16 super-tiles


def _build() -> bass.Bass:
    nc = bass.Bass()
    x = nc.dram_tensor("x", [ROWS, T], mybir.dt.float32, kind="ExternalInput")
    y = nc.dram_tensor("y", [ROWS, T], mybir.dt.float32, kind="ExternalOutput")

    # Super-tile view: row r = (n*K_SUP + j)*128 + p  ->  [n, p, j, t]
    x_r = x.rearrange("(n j p) t -> n p j t", j=K_SUP, p=P)
    y_r = y.rearrange("(n j p) t -> n p j t", j=K_SUP, p=P)

    with TileContext(nc) as tc:
        with (
            tc.tile_pool(name="const", bufs=1) as cpool,
            tc.tile_pool(name="io", bufs=3) as pool,
        ):
            tau = cpool.tile([P, T], mybir.dt.float32)
            nc.vector.memset(tau[:], TAU)
            for i in range(N_SUP):
                xin = pool.tile([P, K_SUP, T], mybir.dt.float32)
                nc.sync.dma_start(out=xin[:], in_=x_r[i])
                uout = pool.tile([P, K_SUP, T], mybir.dt.float32)
                for j in range(K_SUP):
                    nc.vector.tensor_tensor_scan(
                        uout[:, j, :],
                        tau[:],
                        xin[:, j, :],
                        0.0,
                        mybir.AluOpType.mult,
                        mybir.AluOpType.add,
                    )
                nc.sync.dma_start(out=y_r[i], in_=uout[:])

    _split_excess_waits(nc)
    return nc


def kernel(x: np.ndarray, **_unused) -> np.ndarray:
    global _nc_cache, last_results
    if _nc_cache is None:
        _nc_cache = _build()
    nc = _nc_cache

    x = np.ascontiguousarray(np.asarray(x), dtype=np.float32)
    assert x.shape == (B, F, T), x.shape
    shards = [
        np.ascontiguousarray(
            x[c * B_PER_CORE : (c + 1) * B_PER_CORE].reshape(ROWS, T)
        )
        for c in range(N_CORES)
    ]
    last_results = run_bass_kernel_spmd(
        nc, [{"x": s} for s in shards], core_ids=list(range(N_CORES))
    )
    out = np.concatenate(
        [r["y"].reshape(B_PER_CORE, F, T) for r in last_results.results], axis=0
    )
    return out



# revision 2
# speedup vs baseline: 1.6225x; 1.6225x over previous
"""Leaky-integrator linear recurrence kernel for Trainium2.

u_t = TAU * u_{t-1} + x_t along the last (time) axis of x[32, 1024, 2048] f32.

Strategy: data-parallel across 8 NeuronCores (4 batches each). The problem is
memory-bound (the scan itself is cheap), so HBM traffic is halved by moving
data as fp16: the host casts x to fp16, the device computes the scan with fp32
internal state (TensorTensorScanArith keeps fp32 state regardless of operand
dtype) and writes fp16, and the host upcasts the result. The fp16
quantization error (~5e-4 relative) is far inside the 2e-2 tolerance.

Per core the shard is [4096 rows, 2048 time]. Each SBUF tile packs
R_PER_P=4 consecutive rows per partition ([128, 4*2048] fp16), which makes
every partition line one 16 KiB contiguous HBM run (large DMA descriptors at
line rate). The recurrence runs along the free dimension via the Vector
engine's hardware scan (state = tau*state + x), one scan per row-segment.

Input DMAs issue from nc.sync (SP) and output DMAs from nc.scalar (ACT): two
HWDGE rings, so the DMA engines round-robin between input prefetch and output
drain instead of head-of-line blocking on a single queue.

The walrus build in this container allows at most ONE embedded sync-wait
per engine instruction (two on EventSemaphore); Tile's wait assignment can
attach several. _split_excess_waits() hoists the extras onto standalone
EventSemaphore instructions inserted immediately before, on the same
engine — conservative (the engine waits a bit earlier than strictly
needed) but correct, since every awaited semaphore's producer precedes the
waiter in the scheduled program order.
"""

import numpy as np

import concourse.bass as bass
import concourse.mybir as mybir
from concourse.bass_utils import run_bass_kernel_spmd
from concourse.tile import TileContext

TAU = 0.9
B, F, T = 32, 1024, 2048
N_CORES = 8
B_PER_CORE = B // N_CORES          # 4
ROWS = B_PER_CORE * F              # 4096 independent recurrences per core
P = 128
R_PER_P = 4                        # rows packed per partition line
SEG = R_PER_P * T                  # free elements per partition per tile
N_TILES = ROWS // (P * R_PER_P)    # 8 tiles of [128, 4*2048]

_nc_cache = None
last_results = None  # BassKernelResults from the most recent run (for test.py)


def _split_excess_waits(nc: bass.Bass) -> None:
    for fn in nc.m.functions:
        for blk in fn.blocks:
            out = []
            changed = False
            for inst in blk.instructions:
                si = inst.sync_info
                waits = list(si.on_wait) if si is not None else []
                cap = 2 if inst.opcode == "EventSemaphore" else 1
                if len(waits) <= cap:
                    out.append(inst)
                    continue
                changed = True
                # On DMAs keep a queue-ordering (DMAHW*) wait embedded so
                # queue-level throttling stays at the queue; otherwise keep
                # the last wait.
                keep_idx = len(waits) - 1
                if inst.opcode == "DMACopy":
                    for k, w in enumerate(waits):
                        if (w.ant_name or "").startswith("DMA"):
                            keep_idx = k
                            break
                rest = [w for j, w in enumerate(waits) if j != keep_idx]
                for j in range(0, len(rest), 2):
                    out.append(
                        mybir.InstEventSemaphore(
                            name=f"{inst.name}-xw{j}",
                            opcode="EventSemaphore",
                            engine=inst.engine,
                            debug=inst.debug,
                            sync_info=mybir.SyncInfo(
                                on_wait=rest[j : j + 2], on_update=[]
                            ),
                        )
                    )
                inst.sync_info = mybir.SyncInfo(
                    on_wait=[waits[keep_idx]], on_update=list(si.on_update)
                )
                out.append(inst)
            if changed:
                blk.instructions = out


def _build() -> bass.Bass:
    nc = bass.Bass()
    x = nc.dram_tensor("x", [ROWS, T], mybir.dt.float16, kind="ExternalInput")
    y = nc.dram_tensor("y", [ROWS, T], mybir.dt.float16, kind="ExternalOutput")

    # Tile view: row = i*(P*R_PER_P) + p*R_PER_P + s  ->  [i, p, (s t)]
    # (s t) is contiguous in HBM: one 16 KiB run per partition line.
    x_r = x.rearrange("(i p s) t -> i p (s t)", p=P, s=R_PER_P)
    y_r = y.rearrange("(i p s) t -> i p (s t)", p=P, s=R_PER_P)

    with TileContext(nc) as tc:
        with (
            tc.tile_pool(name="const", bufs=1) as cpool,
            tc.tile_pool(name="in", bufs=4) as ipool,
            tc.tile_pool(name="out", bufs=4) as opool,
        ):
            tau = cpool.tile([P, T], mybir.dt.float16)
            nc.vector.memset(tau[:], TAU)
            for i in range(N_TILES):
                xin = ipool.tile([P, SEG], mybir.dt.float16)
                nc.sync.dma_start(out=xin[:], in_=x_r[i])
                uout = opool.tile([P, SEG], mybir.dt.float16)
                for r in range(R_PER_P):
                    nc.vector.tensor_tensor_scan(
                        uout[:, r * T : (r + 1) * T],
                        tau[:],
                        xin[:, r * T : (r + 1) * T],
                        0.0,
                        mybir.AluOpType.mult,
                        mybir.AluOpType.add,
                    )
                nc.scalar.dma_start(out=y_r[i], in_=uout[:])

    _split_excess_waits(nc)
    return nc


def kernel(x: np.ndarray, **_unused) -> np.ndarray:
    global _nc_cache, last_results
    if _nc_cache is None:
        _nc_cache = _build()
    nc = _nc_cache

    x = np.asarray(x)
    assert x.shape == (B, F, T), x.shape
    x16 = np.ascontiguousarray(x, dtype=np.float16)
    shards = [
        np.ascontiguousarray(
            x16[c * B_PER_CORE : (c + 1) * B_PER_CORE].reshape(ROWS, T)
        )
        for c in range(N_CORES)
    ]
    last_results = run_bass_kernel_spmd(
        nc, [{"x": s} for s in shards], core_ids=list(range(N_CORES))
    )
    out = np.concatenate(
        [
            r["y"].astype(np.float32).reshape(B_PER_CORE, F, T)
            for r in last_results.results
        ],
        axis=0,
    )
    return out


# revision 4
# speedup vs baseline: 2.1762x; 1.3413x over previous
"""Leaky-integrator linear recurrence kernel for Trainium2.

u_t = TAU * u_{t-1} + x_t along the last (time) axis of x[32, 1024, 2048] f32.

Strategy: data-parallel across 8 NeuronCores (4 batches each). The problem is
memory-bound, so HBM traffic is halved by moving data as fp16 (the 2e-2
tolerance dwarfs fp16 quantization at ~5e-4). The recurrence itself is
computed on the otherwise-idle Tensor engine as a *banded matmul*: since
TAU^129 < 2e-6, u_t is (to float precision) a windowed sum
u_t = sum_{s=t-255..t} TAU^(t-s) x_s. In a host-transposed layout
xt[time, rows], each 128-step output block i is

    u[i*128+m, r] = sum_{k=0..255} C[k, m] * xpad[i*128+k, r]

with C[k, m] = TAU^(m+128-k) (zero for k > m+128) — two accumulating
128x128-stationary matmuls per PSUM chunk. The host zero-pads 128 time
steps in front so block 0 needs no special case.

Engine assignment: Sync issues input DMAs, Scalar issues output DMAs (two
HWDGE rings, so input prefetch never head-of-line blocks behind output
drain), Tensor does the matmuls, Vector downcasts PSUM f32 -> SBUF fp16.
The Vector scan version of this kernel ran at 2 cycles/element (no 16-bit
perf mode for TensorTensorScanArith), ~138 us of Vector busy; the matmul
formulation moves that work to TensorE at ~55 us and leaves DMA as the
binding constraint (~33.8 MB/core at ~358 GB/s ≈ 95 us).

The walrus build in this container allows at most ONE embedded sync-wait
per engine instruction (two on EventSemaphore); Tile's wait assignment can
attach several. _split_excess_waits() hoists the extras onto standalone
EventSemaphore instructions inserted immediately before, on the same
engine — conservative but correct, since every awaited semaphore's
producer precedes the waiter in the scheduled program order.
"""

import numpy as np

import concourse.bass as bass
import concourse.mybir as mybir
from concourse.bass_utils import run_bass_kernel_spmd
from concourse.tile import TileContext

TAU = 0.9
B, F, T = 32, 1024, 2048
N_CORES = 8
B_PER_CORE = B // N_CORES          # 4
ROWS = B_PER_CORE * F              # 4096 independent recurrences per core
P = 128
N_BLK = T // P                     # 16 output time-blocks per core
T_PAD = T + P                      # zero-padded time length (17 slabs)
N_SLAB = T_PAD // P                # 17
CHUNK = 512                        # PSUM bank width (f32)
N_CHUNK = ROWS // CHUNK            # 8

_nc_cache = None
_coef_cache = None
last_results = None  # BassKernelResults from the most recent run (for test.py)


def _split_excess_waits(nc: bass.Bass) -> None:
    for fn in nc.m.functions:
        for blk in fn.blocks:
            out = []
            changed = False
            for inst in blk.instructions:
                si = inst.sync_info
                waits = list(si.on_wait) if si is not None else []
                cap = 2 if inst.opcode == "EventSemaphore" else 1
                if len(waits) <= cap:
                    out.append(inst)
                    continue
                changed = True
                # On DMAs keep a queue-ordering (DMAHW*) wait embedded so
                # queue-level throttling stays at the queue; otherwise keep
                # the last wait.
                keep_idx = len(waits) - 1
                if inst.opcode == "DMACopy":
                    for k, w in enumerate(waits):
                        if (w.ant_name or "").startswith("DMA"):
                            keep_idx = k
                            break
                rest = [w for j, w in enumerate(waits) if j != keep_idx]
                for j in range(0, len(rest), 2):
                    out.append(
                        mybir.InstEventSemaphore(
                            name=f"{inst.name}-xw{j}",
                            opcode="EventSemaphore",
                            engine=inst.engine,
                            debug=inst.debug,
                            sync_info=mybir.SyncInfo(
                                on_wait=rest[j : j + 2], on_update=[]
                            ),
                        )
                    )
                inst.sync_info = mybir.SyncInfo(
                    on_wait=[waits[keep_idx]], on_update=list(si.on_update)
                )
                out.append(inst)
            if changed:
                blk.instructions = out


def _coef() -> np.ndarray:
    # C[k, m] = TAU^(m+128-k) for k <= m+128 else 0;  k in [0,256), m in [0,128)
    k = np.arange(2 * P)[:, None]
    m = np.arange(P)[None, :]
    e = m + P - k
    c = np.where(e >= 0, TAU ** np.maximum(e, 0).astype(np.float64), 0.0)
    return c.astype(np.float16)


def _build() -> bass.Bass:
    nc = bass.Bass()
    xt = nc.dram_tensor("xt", [T_PAD, ROWS], mybir.dt.float16, kind="ExternalInput")
    coef = nc.dram_tensor("coef", [2 * P, P], mybir.dt.float16, kind="ExternalInput")
    yt = nc.dram_tensor("yt", [T, ROWS], mybir.dt.float16, kind="ExternalOutput")

    x_r = xt.rearrange("(i p) r -> i p r", p=P)   # 17 slabs [128, ROWS]
    y_r = yt.rearrange("(i p) r -> i p r", p=P)   # 16 blocks [128, ROWS]

    with TileContext(nc) as tc:
        with (
            tc.tile_pool(name="const", bufs=1) as cpool,
            tc.tile_pool(name="in", bufs=6) as ipool,
            tc.tile_pool(name="out", bufs=4) as opool,
            tc.tile_pool(name="psum", bufs=8, space="PSUM") as ppool,
        ):
            cA = cpool.tile([P, P], mybir.dt.float16)
            cB = cpool.tile([P, P], mybir.dt.float16)
            nc.sync.dma_start(out=cA[:], in_=coef[0:P, :])
            nc.sync.dma_start(out=cB[:], in_=coef[P : 2 * P, :])

            slabs = []
            for i in range(N_SLAB):
                s = ipool.tile([P, ROWS], mybir.dt.float16)
                nc.sync.dma_start(out=s[:], in_=x_r[i])
                slabs.append(s)

                if i == 0:
                    continue
                blk = i - 1
                utile = opool.tile([P, ROWS], mybir.dt.float16)
                for c in range(N_CHUNK):
                    pt = ppool.tile([P, CHUNK], mybir.dt.float32)
                    sl = slice(c * CHUNK, (c + 1) * CHUNK)
                    nc.tensor.matmul(
                        pt[:], lhsT=cA[:], rhs=slabs[blk][:, sl],
                        start=True, stop=False,
                    )
                    nc.tensor.matmul(
                        pt[:], lhsT=cB[:], rhs=slabs[blk + 1][:, sl],
                        start=False, stop=True,
                    )
                    nc.vector.tensor_copy(utile[:, sl], pt[:])
                nc.scalar.dma_start(out=y_r[blk], in_=utile[:])
                slabs[blk] = None  # release our reference; pool rotation rules

    _split_excess_waits(nc)
    return nc


def kernel(x: np.ndarray, **_unused) -> np.ndarray:
    global _nc_cache, _coef_cache, last_results
    if _nc_cache is None:
        _nc_cache = _build()
        _coef_cache = _coef()
    nc = _nc_cache

    x = np.asarray(x)
    assert x.shape == (B, F, T), x.shape
    x16 = np.ascontiguousarray(x.reshape(N_CORES, ROWS, T), dtype=np.float16)
    in_maps = []
    for c in range(N_CORES):
        xt = np.zeros((T_PAD, ROWS), dtype=np.float16)
        xt[P:] = x16[c].T
        in_maps.append({"xt": xt, "coef": _coef_cache})
    last_results = run_bass_kernel_spmd(
        nc, in_maps, core_ids=list(range(N_CORES))
    )
    out = np.concatenate(
        [
            r["yt"].T.astype(np.float32).reshape(B_PER_CORE, F, T)
            for r in last_results.results
        ],
        axis=0,
    )
    return out


# revision 5
# speedup vs baseline: 2.4206x; 1.1123x over previous
"""Leaky-integrator linear recurrence kernel for Trainium2.

u_t = TAU * u_{t-1} + x_t along the last (time) axis of x[32, 1024, 2048] f32.

Strategy: data-parallel across 8 NeuronCores (4 batches each). The problem is
memory-bound, so HBM traffic is halved by moving data as 16-bit floats (the
2e-2 tolerance dwarfs the quantization error). The recurrence is computed on
the Tensor engine as a *banded matmul*: since TAU^129 < 2e-6, u_t is (to
float precision) a windowed sum u_t = sum_{s=t-255..t} TAU^(t-s) x_s. In a
host-transposed layout xt[time, rows], each 128-step output block i is

    u[i*128+m, r] = sum_{k} A[k, m] * xt[(i-1)*128+k, r]   (cross-block band)
                  + sum_{k} B[k, m] * xt[i*128+k, r]       (triangular band)

with A[k, m] = TAU^(m+128-k), B[k, m] = TAU^(m-k) for k<=m else 0 — two
accumulating 128x128-stationary matmuls per PSUM chunk (block 0 skips A).

Engine assignment: Sync issues input DMAs, Scalar issues output DMAs (two
HWDGE rings, so input prefetch never head-of-line blocks behind output
drain), Tensor does the matmuls, and the PSUM f32 -> SBUF 16-bit downcast
copies are split between Vector and Scalar (each ~46 us; a single engine
at ~92 us would sit on the critical path).

The walrus build in this container allows at most ONE embedded sync-wait
per engine instruction (two on EventSemaphore); Tile's wait assignment can
attach several. _split_excess_waits() hoists the extras onto standalone
EventSemaphore instructions inserted immediately before, on the same
engine — conservative but correct, since every awaited semaphore's
producer precedes the waiter in the scheduled program order.
"""

import numpy as np
import ml_dtypes

import concourse.bass as bass
import concourse.mybir as mybir
from concourse.bass_utils import run_bass_kernel_spmd
from concourse.tile import TileContext

TAU = 0.9
B, F, T = 32, 1024, 2048
N_CORES = 8
B_PER_CORE = B // N_CORES          # 4
ROWS = B_PER_CORE * F              # 4096 independent recurrences per core
P = 128
N_BLK = T // P                     # 16 time-blocks (slabs) per core
CHUNK = 512                        # PSUM bank width (f32)
N_CHUNK = ROWS // CHUNK            # 8

NP_DT = ml_dtypes.bfloat16
MYBIR_DT = mybir.dt.bfloat16

_nc_cache = None
_coef_cache = None
last_results = None  # BassKernelResults from the most recent run (for test.py)


def _split_excess_waits(nc: bass.Bass) -> None:
    for fn in nc.m.functions:
        for blk in fn.blocks:
            out = []
            changed = False
            for inst in blk.instructions:
                si = inst.sync_info
                waits = list(si.on_wait) if si is not None else []
                cap = 2 if inst.opcode == "EventSemaphore" else 1
                if len(waits) <= cap:
                    out.append(inst)
                    continue
                changed = True
                # On DMAs keep a queue-ordering (DMAHW*) wait embedded so
                # queue-level throttling stays at the queue; otherwise keep
                # the last wait.
                keep_idx = len(waits) - 1
                if inst.opcode == "DMACopy":
                    for k, w in enumerate(waits):
                        if (w.ant_name or "").startswith("DMA"):
                            keep_idx = k
                            break
                rest = [w for j, w in enumerate(waits) if j != keep_idx]
                for j in range(0, len(rest), 2):
                    out.append(
                        mybir.InstEventSemaphore(
                            name=f"{inst.name}-xw{j}",
                            opcode="EventSemaphore",
                            engine=inst.engine,
                            debug=inst.debug,
                            sync_info=mybir.SyncInfo(
                                on_wait=rest[j : j + 2], on_update=[]
                            ),
                        )
                    )
                inst.sync_info = mybir.SyncInfo(
                    on_wait=[waits[keep_idx]], on_update=list(si.on_update)
                )
                out.append(inst)
            if changed:
                blk.instructions = out


def _coef() -> np.ndarray:
    # [2P, P]: rows 0..127 = A[k, m] = TAU^(m+128-k) (cross-block band),
    #          rows 128..255 = B[k, m] = TAU^(m-k) for k <= m else 0.
    k = np.arange(2 * P)[:, None]
    m = np.arange(P)[None, :]
    e = m + P - k
    c = np.where(e >= 0, TAU ** np.maximum(e, 0).astype(np.float64), 0.0)
    return c.astype(NP_DT)


def _build() -> bass.Bass:
    nc = bass.Bass()
    xt = nc.dram_tensor("xt", [T, ROWS], MYBIR_DT, kind="ExternalInput")
    coef = nc.dram_tensor("coef", [2 * P, P], MYBIR_DT, kind="ExternalInput")
    yt = nc.dram_tensor("yt", [T, ROWS], MYBIR_DT, kind="ExternalOutput")

    x_r = xt.rearrange("(i p) r -> i p r", p=P)   # 16 slabs [128, ROWS]
    y_r = yt.rearrange("(i p) r -> i p r", p=P)   # 16 blocks [128, ROWS]

    with TileContext(nc) as tc:
        with (
            tc.tile_pool(name="const", bufs=1) as cpool,
            tc.tile_pool(name="in", bufs=8) as ipool,
            tc.tile_pool(name="out", bufs=4) as opool,
            tc.tile_pool(name="psum", bufs=8, space="PSUM") as ppool,
        ):
            cA = cpool.tile([P, P], MYBIR_DT)
            cB = cpool.tile([P, P], MYBIR_DT)
            nc.sync.dma_start(out=cA[:], in_=coef[0:P, :])
            nc.sync.dma_start(out=cB[:], in_=coef[P : 2 * P, :])

            slabs = []
            for i in range(N_BLK):
                s = ipool.tile([P, ROWS], MYBIR_DT)
                nc.sync.dma_start(out=s[:], in_=x_r[i])
                slabs.append(s)

                utile = opool.tile([P, ROWS], MYBIR_DT)
                for c in range(N_CHUNK):
                    pt = ppool.tile([P, CHUNK], mybir.dt.float32)
                    sl = slice(c * CHUNK, (c + 1) * CHUNK)
                    if i == 0:
                        nc.tensor.matmul(
                            pt[:], lhsT=cB[:], rhs=slabs[i][:, sl],
                            start=True, stop=True,
                        )
                    else:
                        nc.tensor.matmul(
                            pt[:], lhsT=cA[:], rhs=slabs[i - 1][:, sl],
                            start=True, stop=False,
                        )
                        nc.tensor.matmul(
                            pt[:], lhsT=cB[:], rhs=slabs[i][:, sl],
                            start=False, stop=True,
                        )
                    if c % 2 == 0:
                        nc.vector.tensor_copy(utile[:, sl], pt[:])
                    else:
                        nc.scalar.copy(utile[:, sl], pt[:])
                nc.scalar.dma_start(out=y_r[i], in_=utile[:])
                if i >= 1:
                    slabs[i - 1] = None

    _split_excess_waits(nc)
    return nc


def kernel(x: np.ndarray, **_unused) -> np.ndarray:
    global _nc_cache, _coef_cache, last_results
    if _nc_cache is None:
        _nc_cache = _build()
        _coef_cache = _coef()
    nc = _nc_cache

    x = np.asarray(x)
    assert x.shape == (B, F, T), x.shape
    x16 = np.ascontiguousarray(x.reshape(N_CORES, ROWS, T), dtype=NP_DT)
    in_maps = [
        {"xt": np.ascontiguousarray(x16[c].T), "coef": _coef_cache}
        for c in range(N_CORES)
    ]
    last_results = run_bass_kernel_spmd(
        nc, in_maps, core_ids=list(range(N_CORES))
    )
    out = np.concatenate(
        [
            r["yt"].T.astype(np.float32).reshape(B_PER_CORE, F, T)
            for r in last_results.results
        ],
        axis=0,
    )
    return out


# revision 8
# speedup vs baseline: 2.4543x; 1.0139x over previous
"""Leaky-integrator linear recurrence kernel for Trainium2.

u_t = TAU * u_{t-1} + x_t along the last (time) axis of x[32, 1024, 2048] f32.

Strategy: data-parallel across 8 NeuronCores (4 batches each). The problem is
memory-bound, so HBM traffic is halved by moving data as 16-bit floats (the
2e-2 tolerance dwarfs the quantization error). The recurrence is computed on
the Tensor engine as a *banded matmul*: since TAU^129 < 2e-6, u_t is (to
float precision) a windowed sum u_t = sum_{s=t-255..t} TAU^(t-s) x_s. In a
host-transposed layout xt[time, rows], each 128-step output block i is

    u[i*128+m, r] = sum_{k} A[k, m] * xt[(i-1)*128+k, r]   (cross-block band)
                  + sum_{k} B[k, m] * xt[i*128+k, r]       (triangular band)

with A[k, m] = TAU^(m+128-k), B[k, m] = TAU^(m-k) for k<=m else 0 — two
accumulating 128x128-stationary matmuls per PSUM chunk (block 0 skips A).

Engine assignment: Sync issues input DMAs, Scalar issues output DMAs (two
HWDGE rings, so input prefetch never head-of-line blocks behind output
drain), Tensor does the matmuls, and the PSUM f32 -> SBUF 16-bit downcast
copies are split between Vector and Scalar (each ~46 us; a single engine
at ~92 us would sit on the critical path).

The walrus build in this container allows at most ONE embedded sync-wait
per engine instruction (two on EventSemaphore); Tile's wait assignment can
attach several. _split_excess_waits() hoists the extras onto standalone
EventSemaphore instructions inserted immediately before, on the same
engine — conservative but correct, since every awaited semaphore's
producer precedes the waiter in the scheduled program order.
"""

import numpy as np
import ml_dtypes

import concourse.bass as bass
import concourse.mybir as mybir
from concourse.bass_utils import run_bass_kernel_spmd
from concourse.tile import TileContext

TAU = 0.9
B, F, T = 32, 1024, 2048
N_CORES = 8
B_PER_CORE = B // N_CORES          # 4
ROWS = B_PER_CORE * F              # 4096 independent recurrences per core
P = 128
N_BLK = T // P                     # 16 time-blocks (slabs) per core
CHUNK = 512                        # PSUM bank width (f32)
N_CHUNK = ROWS // CHUNK            # 8

NP_DT = ml_dtypes.bfloat16
MYBIR_DT = mybir.dt.bfloat16

_nc_cache = None
_coef_cache = None
last_results = None  # BassKernelResults from the most recent run (for test.py)


def _split_excess_waits(nc: bass.Bass) -> None:
    for fn in nc.m.functions:
        for blk in fn.blocks:
            out = []
            changed = False
            for inst in blk.instructions:
                si = inst.sync_info
                waits = list(si.on_wait) if si is not None else []
                cap = 2 if inst.opcode == "EventSemaphore" else 1
                if len(waits) <= cap:
                    out.append(inst)
                    continue
                changed = True
                # On DMAs keep a queue-ordering (DMAHW*) wait embedded so
                # queue-level throttling stays at the queue; otherwise keep
                # the last wait.
                keep_idx = len(waits) - 1
                if inst.opcode == "DMACopy":
                    for k, w in enumerate(waits):
                        if (w.ant_name or "").startswith("DMA"):
                            keep_idx = k
                            break
                rest = [w for j, w in enumerate(waits) if j != keep_idx]
                for j in range(0, len(rest), 2):
                    out.append(
                        mybir.InstEventSemaphore(
                            name=f"{inst.name}-xw{j}",
                            opcode="EventSemaphore",
                            engine=inst.engine,
                            debug=inst.debug,
                            sync_info=mybir.SyncInfo(
                                on_wait=rest[j : j + 2], on_update=[]
                            ),
                        )
                    )
                inst.sync_info = mybir.SyncInfo(
                    on_wait=[waits[keep_idx]], on_update=list(si.on_update)
                )
                out.append(inst)
            if changed:
                blk.instructions = out


def _coef() -> np.ndarray:
    # [P, 2P] = [A | B] packed side by side (one SBUF tile, one DMA):
    #   A[k, m] = TAU^(m+128-k)                (cross-block band)
    #   B[k, m] = TAU^(m-k) for k <= m else 0  (triangular band)
    k = np.arange(2 * P)[:, None]
    m = np.arange(P)[None, :]
    e = m + P - k
    c = np.where(e >= 0, TAU ** np.maximum(e, 0).astype(np.float64), 0.0)
    return np.ascontiguousarray(
        np.hstack([c[:P], c[P:]]).astype(NP_DT)
    )


def _build() -> bass.Bass:
    nc = bass.Bass()
    xt = nc.dram_tensor("xt", [T, ROWS], MYBIR_DT, kind="ExternalInput")
    coef = nc.dram_tensor("coef", [P, 2 * P], MYBIR_DT, kind="ExternalInput")
    yt = nc.dram_tensor("yt", [T, ROWS], MYBIR_DT, kind="ExternalOutput")

    x_r = xt.rearrange("(i p) r -> i p r", p=P)   # 16 slabs [128, ROWS]
    y_r = yt.rearrange("(i p) r -> i p r", p=P)   # 16 blocks [128, ROWS]

    with TileContext(nc) as tc:
        with (
            tc.tile_pool(name="const", bufs=1) as cpool,
            tc.tile_pool(name="in", bufs=8) as ipool,
            tc.tile_pool(name="out", bufs=4) as opool,
            tc.tile_pool(name="psum", bufs=8, space="PSUM") as ppool,
        ):
            cf = cpool.tile([P, 2 * P], MYBIR_DT)
            nc.sync.dma_start(out=cf[:], in_=coef[:])
            cA = cf[:, 0:P]
            cB = cf[:, P : 2 * P]

            LAST = N_BLK - 1
            slabs = []
            for i in range(N_BLK):
                s = ipool.tile([P, ROWS], MYBIR_DT)
                if i == LAST:
                    # Final block: half-granular input and quarter-granular
                    # output so its writes are ready as the read stream ends
                    # (shortens the exposed tail chain).
                    h = ROWS // 2
                    nc.sync.dma_start(out=s[:, 0:h], in_=x_r[i][:, 0:h])
                    nc.sync.dma_start(out=s[:, h:ROWS], in_=x_r[i][:, h:ROWS])
                else:
                    nc.sync.dma_start(out=s[:], in_=x_r[i])
                slabs.append(s)

                utile = opool.tile([P, ROWS], MYBIR_DT)
                for c in range(N_CHUNK):
                    pt = ppool.tile([P, CHUNK], mybir.dt.float32)
                    sl = slice(c * CHUNK, (c + 1) * CHUNK)
                    if i == 0:
                        nc.tensor.matmul(
                            pt[:], lhsT=cB[:], rhs=slabs[i][:, sl],
                            start=True, stop=True,
                        )
                    else:
                        nc.tensor.matmul(
                            pt[:], lhsT=cA[:], rhs=slabs[i - 1][:, sl],
                            start=True, stop=False,
                        )
                        nc.tensor.matmul(
                            pt[:], lhsT=cB[:], rhs=slabs[i][:, sl],
                            start=False, stop=True,
                        )
                    if c % 2 == 0:
                        nc.vector.tensor_copy(utile[:, sl], pt[:])
                    else:
                        nc.scalar.copy(utile[:, sl], pt[:])
                    if i == LAST and c % 2 == 1:
                        qs = slice((c - 1) * CHUNK, (c + 1) * CHUNK)
                        nc.scalar.dma_start(out=y_r[i][:, qs], in_=utile[:, qs])
                if i != LAST:
                    nc.scalar.dma_start(out=y_r[i], in_=utile[:])
                if i >= 1:
                    slabs[i - 1] = None

    _split_excess_waits(nc)
    return nc


def kernel(x: np.ndarray, **_unused) -> np.ndarray:
    global _nc_cache, _coef_cache, last_results
    if _nc_cache is None:
        _nc_cache = _build()
        _coef_cache = _coef()
    nc = _nc_cache

    x = np.asarray(x)
    assert x.shape == (B, F, T), x.shape
    x16 = np.ascontiguousarray(x.reshape(N_CORES, ROWS, T), dtype=NP_DT)
    in_maps = [
        {"xt": np.ascontiguousarray(x16[c].T), "coef": _coef_cache}
        for c in range(N_CORES)
    ]
    last_results = run_bass_kernel_spmd(
        nc, in_maps, core_ids=list(range(N_CORES))
    )
    out = np.concatenate(
        [
            r["yt"].T.astype(np.float32).reshape(B_PER_CORE, F, T)
            for r in last_results.results
        ],
        axis=0,
    )
    return out


# revision 9
# speedup vs baseline: 2.5586x; 1.0425x over previous
"""Leaky-integrator linear recurrence kernel for Trainium2.

u_t = TAU * u_{t-1} + x_t along the last (time) axis of x[32, 1024, 2048] f32.

Strategy: data-parallel across 8 NeuronCores (4 batches each). The problem is
memory-bound, so HBM traffic is halved by moving data as 16-bit floats (the
2e-2 tolerance dwarfs the quantization error). The recurrence is computed on
the Tensor engine as a *banded matmul*: since TAU^129 < 2e-6, u_t is (to
float precision) a windowed sum u_t = sum_{s=t-255..t} TAU^(t-s) x_s. In a
host-transposed layout xt[time, rows], each 128-step output block i is

    u[i*128+m, r] = sum_{k} A[k, m] * xt[(i-1)*128+k, r]   (cross-block band)
                  + sum_{k} B[k, m] * xt[i*128+k, r]       (triangular band)

with A[k, m] = TAU^(m+128-k), B[k, m] = TAU^(m-k) for k<=m else 0 — two
accumulating 128x128-stationary matmuls per PSUM chunk (block 0 skips A).

Engine assignment: Sync issues input DMAs, Scalar issues output DMAs (two
HWDGE rings, so input prefetch never head-of-line blocks behind output
drain), Tensor does the matmuls, and the PSUM f32 -> SBUF 16-bit downcast
copies are split between Vector and Scalar (each ~46 us; a single engine
at ~92 us would sit on the critical path).

The walrus build in this container allows at most ONE embedded sync-wait
per engine instruction (two on EventSemaphore); Tile's wait assignment can
attach several. _split_excess_waits() hoists the extras onto standalone
EventSemaphore instructions inserted immediately before, on the same
engine — conservative but correct, since every awaited semaphore's
producer precedes the waiter in the scheduled program order.
"""

import numpy as np
import ml_dtypes

import concourse.bass as bass
import concourse.mybir as mybir
from concourse.bass_utils import run_bass_kernel_spmd
from concourse.tile import TileContext

TAU = 0.9
B, F, T = 32, 1024, 2048
N_CORES = 8
B_PER_CORE = B // N_CORES          # 4
ROWS = B_PER_CORE * F              # 4096 independent recurrences per core
P = 128
N_BLK = T // P                     # 16 time-blocks (slabs) per core
CHUNK = 512                        # PSUM bank width (f32)
N_CHUNK = ROWS // CHUNK            # 8

NP_DT = ml_dtypes.bfloat16
MYBIR_DT = mybir.dt.bfloat16

_nc_cache = None
_coef_cache = None
last_results = None  # BassKernelResults from the most recent run (for test.py)


def _split_excess_waits(nc: bass.Bass) -> None:
    for fn in nc.m.functions:
        for blk in fn.blocks:
            out = []
            changed = False
            for inst in blk.instructions:
                si = inst.sync_info
                waits = list(si.on_wait) if si is not None else []
                cap = 2 if inst.opcode == "EventSemaphore" else 1
                if len(waits) <= cap:
                    out.append(inst)
                    continue
                changed = True
                # On DMAs keep a queue-ordering (DMAHW*) wait embedded so
                # queue-level throttling stays at the queue; otherwise keep
                # the last wait.
                keep_idx = len(waits) - 1
                if inst.opcode == "DMACopy":
                    for k, w in enumerate(waits):
                        if (w.ant_name or "").startswith("DMA"):
                            keep_idx = k
                            break
                rest = [w for j, w in enumerate(waits) if j != keep_idx]
                for j in range(0, len(rest), 2):
                    out.append(
                        mybir.InstEventSemaphore(
                            name=f"{inst.name}-xw{j}",
                            opcode="EventSemaphore",
                            engine=inst.engine,
                            debug=inst.debug,
                            sync_info=mybir.SyncInfo(
                                on_wait=rest[j : j + 2], on_update=[]
                            ),
                        )
                    )
                inst.sync_info = mybir.SyncInfo(
                    on_wait=[waits[keep_idx]], on_update=list(si.on_update)
                )
                out.append(inst)
            if changed:
                blk.instructions = out


def _coef() -> np.ndarray:
    # [P, 2P] = [A | B] packed side by side (one SBUF tile, one DMA):
    #   A[k, m] = TAU^(m+128-k)                (cross-block band)
    #   B[k, m] = TAU^(m-k) for k <= m else 0  (triangular band)
    k = np.arange(2 * P)[:, None]
    m = np.arange(P)[None, :]
    e = m + P - k
    c = np.where(e >= 0, TAU ** np.maximum(e, 0).astype(np.float64), 0.0)
    return np.ascontiguousarray(
        np.hstack([c[:P], c[P:]]).astype(NP_DT)
    )


def _build() -> bass.Bass:
    nc = bass.Bass()
    xt = nc.dram_tensor("xt", [T, ROWS], MYBIR_DT, kind="ExternalInput")
    coef = nc.dram_tensor("coef", [P, 2 * P], MYBIR_DT, kind="ExternalInput")
    yt = nc.dram_tensor("yt", [T, ROWS], MYBIR_DT, kind="ExternalOutput")

    x_r = xt.rearrange("(i p) r -> i p r", p=P)   # 16 slabs [128, ROWS]
    y_r = yt.rearrange("(i p) r -> i p r", p=P)   # 16 blocks [128, ROWS]

    with TileContext(nc) as tc:
        with (
            tc.tile_pool(name="const", bufs=1) as cpool,
            tc.tile_pool(name="in", bufs=8) as ipool,
            tc.tile_pool(name="out", bufs=4) as opool,
            tc.tile_pool(name="psum", bufs=8, space="PSUM") as ppool,
        ):
            cf = cpool.tile([P, 2 * P], MYBIR_DT)
            nc.sync.dma_start(out=cf[:], in_=coef[:])
            cA = cf[:, 0:P]
            cB = cf[:, P : 2 * P]

            LAST = N_BLK - 1
            slabs = []
            for i in range(N_BLK):
                s = ipool.tile([P, ROWS], MYBIR_DT)
                if i == LAST:
                    # Final block: half-granular input and quarter-granular
                    # output so its writes are ready as the read stream ends
                    # (shortens the exposed tail chain).
                    h = ROWS // 2
                    nc.sync.dma_start(out=s[:, 0:h], in_=x_r[i][:, 0:h])
                    nc.sync.dma_start(out=s[:, h:ROWS], in_=x_r[i][:, h:ROWS])
                else:
                    nc.sync.dma_start(out=s[:], in_=x_r[i])
                slabs.append(s)

                utile = opool.tile([P, ROWS], MYBIR_DT)
                # All-A then all-B: consecutive matmuls share the stationary
                # (walrus can skip redundant LDWEIGHTS), and the 8 chunks
                # exactly fill the 8 PSUM banks.
                pts = []
                for c in range(N_CHUNK):
                    pt = ppool.tile([P, CHUNK], mybir.dt.float32)
                    pts.append(pt)
                    sl = slice(c * CHUNK, (c + 1) * CHUNK)
                    if i > 0:
                        nc.tensor.matmul(
                            pt[:], lhsT=cA[:], rhs=slabs[i - 1][:, sl],
                            start=True, stop=False,
                        )
                for c in range(N_CHUNK):
                    sl = slice(c * CHUNK, (c + 1) * CHUNK)
                    nc.tensor.matmul(
                        pts[c][:], lhsT=cB[:], rhs=slabs[i][:, sl],
                        start=(i == 0), stop=True,
                    )
                    if c % 2 == 0:
                        nc.vector.tensor_copy(utile[:, sl], pts[c][:])
                    else:
                        nc.scalar.copy(utile[:, sl], pts[c][:])
                    if i == LAST and c % 2 == 1:
                        qs = slice((c - 1) * CHUNK, (c + 1) * CHUNK)
                        nc.scalar.dma_start(out=y_r[i][:, qs], in_=utile[:, qs])
                if i != LAST:
                    nc.scalar.dma_start(out=y_r[i], in_=utile[:])
                if i >= 1:
                    slabs[i - 1] = None

    _split_excess_waits(nc)
    return nc


def kernel(x: np.ndarray, **_unused) -> np.ndarray:
    global _nc_cache, _coef_cache, last_results
    if _nc_cache is None:
        _nc_cache = _build()
        _coef_cache = _coef()
    nc = _nc_cache

    x = np.asarray(x)
    assert x.shape == (B, F, T), x.shape
    x16 = np.ascontiguousarray(x.reshape(N_CORES, ROWS, T), dtype=NP_DT)
    in_maps = [
        {"xt": np.ascontiguousarray(x16[c].T), "coef": _coef_cache}
        for c in range(N_CORES)
    ]
    last_results = run_bass_kernel_spmd(
        nc, in_maps, core_ids=list(range(N_CORES))
    )
    out = np.concatenate(
        [
            r["yt"].T.astype(np.float32).reshape(B_PER_CORE, F, T)
            for r in last_results.results
        ],
        axis=0,
    )
    return out


# revision 13
# speedup vs baseline: 2.6474x; 1.0347x over previous
"""Leaky-integrator linear recurrence kernel for Trainium2.

u_t = TAU * u_{t-1} + x_t along the last (time) axis of x[32, 1024, 2048] f32.

Strategy: data-parallel across 8 NeuronCores (4 batches each). The problem is
memory-bound, so HBM traffic is halved by moving data as 16-bit floats (the
2e-2 tolerance dwarfs the quantization error). The recurrence is computed on
the Tensor engine as a *banded matmul*: since TAU^129 < 2e-6, u_t is (to
float precision) a windowed sum u_t = sum_{s=t-255..t} TAU^(t-s) x_s. In a
host-transposed layout xt[time, rows], each 128-step output block i is

    u[i*128+m, r] = sum_{k} A[k, m] * xt[(i-1)*128+k, r]   (cross-block band)
                  + sum_{k} B[k, m] * xt[i*128+k, r]       (triangular band)

with A[k, m] = TAU^(m+128-k), B[k, m] = TAU^(m-k) for k<=m else 0 — two
accumulating 128x128-stationary matmuls per PSUM chunk (block 0 skips A).

Engine assignment: Sync issues input DMAs, Scalar issues output DMAs (two
HWDGE rings, so input prefetch never head-of-line blocks behind output
drain), Tensor does the matmuls, and the PSUM f32 -> SBUF 16-bit downcast
copies are split between Vector and Scalar (each ~46 us; a single engine
at ~92 us would sit on the critical path).

The walrus build in this container allows at most ONE embedded sync-wait
per engine instruction (two on EventSemaphore); Tile's wait assignment can
attach several. _split_excess_waits() hoists the extras onto standalone
EventSemaphore instructions inserted immediately before, on the same
engine — conservative but correct, since every awaited semaphore's
producer precedes the waiter in the scheduled program order.
"""

import numpy as np
import ml_dtypes

import concourse.bass as bass
import concourse.mybir as mybir
from concourse.bass_utils import run_bass_kernel_spmd
from concourse.tile import TileContext

TAU = 0.9
B, F, T = 32, 1024, 2048
N_CORES = 8
B_PER_CORE = B // N_CORES          # 4
ROWS = B_PER_CORE * F              # 4096 independent recurrences per core
P = 128
N_BLK = T // P                     # 16 time-blocks (slabs) per core
CHUNK = 512                        # PSUM bank width (f32)
N_CHUNK = ROWS // CHUNK            # 8

NP_DT = ml_dtypes.bfloat16
MYBIR_DT = mybir.dt.bfloat16

_nc_cache = None
_coef_cache = None
last_results = None  # BassKernelResults from the most recent run (for test.py)


def _split_excess_waits(nc: bass.Bass) -> None:
    for fn in nc.m.functions:
        for blk in fn.blocks:
            out = []
            changed = False
            for inst in blk.instructions:
                si = inst.sync_info
                waits = list(si.on_wait) if si is not None else []
                cap = 2 if inst.opcode == "EventSemaphore" else 1
                if len(waits) <= cap:
                    out.append(inst)
                    continue
                changed = True
                # On DMAs keep a queue-ordering (DMAHW*) wait embedded so
                # queue-level throttling stays at the queue; otherwise keep
                # the last wait.
                keep_idx = len(waits) - 1
                if inst.opcode == "DMACopy":
                    for k, w in enumerate(waits):
                        if (w.ant_name or "").startswith("DMA"):
                            keep_idx = k
                            break
                rest = [w for j, w in enumerate(waits) if j != keep_idx]
                for j in range(0, len(rest), 2):
                    out.append(
                        mybir.InstEventSemaphore(
                            name=f"{inst.name}-xw{j}",
                            opcode="EventSemaphore",
                            engine=inst.engine,
                            debug=inst.debug,
                            sync_info=mybir.SyncInfo(
                                on_wait=rest[j : j + 2], on_update=[]
                            ),
                        )
                    )
                inst.sync_info = mybir.SyncInfo(
                    on_wait=[waits[keep_idx]], on_update=list(si.on_update)
                )
                out.append(inst)
            if changed:
                blk.instructions = out


def _dedup_ldweights(nc: bass.Bass) -> None:
    """Drop PE weight reloads that reload the already-loaded stationary.

    tile_legalize splits every matmul into InstLdweights + a
    non-self-loading InstMatmult. Matmult does not clobber the PE weight
    array, so consecutive Ldweights with an identical weights AP are
    redundant — all but the first can go (saving ~100 ns of PE time each,
    ~21 us total here). A redundant Ldweights that carries semaphore
    waits/updates is replaced by an EventSemaphore on the same engine so
    the synchronization is preserved; any other PE instruction resets the
    tracked signature (conservative).
    """
    for fn in nc.m.functions:
        for blk in fn.blocks:
            out = []
            changed = False
            last_sig = None
            for inst in blk.instructions:
                if inst.opcode == "Matmult":
                    out.append(inst)
                    continue
                if inst.opcode != "Ldweights":
                    if inst.engine == mybir.EngineType.PE and inst.opcode not in (
                        "EventSemaphore",
                    ):
                        last_sig = None
                    out.append(inst)
                    continue
                a = inst.ins[0]
                sig = (a.memref, a.offset, str(a.ap), str(a.dtype))
                if sig != last_sig:
                    last_sig = sig
                    out.append(inst)
                    continue
                changed = True
                si = inst.sync_info
                waits = list(si.on_wait) if si is not None else []
                upds = list(si.on_update) if si is not None else []
                if waits or upds:
                    for j in range(0, max(len(waits), 1), 2):
                        out.append(
                            mybir.InstEventSemaphore(
                                name=f"{inst.name}-lw{j}",
                                opcode="EventSemaphore",
                                engine=inst.engine,
                                debug=inst.debug,
                                sync_info=mybir.SyncInfo(
                                    on_wait=waits[j : j + 2],
                                    on_update=upds if j == 0 else [],
                                ),
                            )
                        )
            if changed:
                blk.instructions = out


def _coef() -> np.ndarray:
    # [P, 2P] = [A | B] packed side by side (one SBUF tile, one DMA):
    #   A[k, m] = TAU^(m+128-k)                (cross-block band)
    #   B[k, m] = TAU^(m-k) for k <= m else 0  (triangular band)
    k = np.arange(2 * P)[:, None]
    m = np.arange(P)[None, :]
    e = m + P - k
    c = np.where(e >= 0, TAU ** np.maximum(e, 0).astype(np.float64), 0.0)
    return np.ascontiguousarray(
        np.hstack([c[:P], c[P:]]).astype(NP_DT)
    )


def _build() -> bass.Bass:
    nc = bass.Bass()
    xt = nc.dram_tensor("xt", [T, ROWS], MYBIR_DT, kind="ExternalInput")
    coef = nc.dram_tensor("coef", [P, 2 * P], MYBIR_DT, kind="ExternalInput")
    yt = nc.dram_tensor("yt", [T, ROWS], MYBIR_DT, kind="ExternalOutput")

    x_r = xt.rearrange("(i p) r -> i p r", p=P)   # 16 slabs [128, ROWS]
    y_r = yt.rearrange("(i p) r -> i p r", p=P)   # 16 blocks [128, ROWS]

    with TileContext(nc) as tc:
        with (
            tc.tile_pool(name="const", bufs=1) as cpool,
            tc.tile_pool(name="in", bufs=8) as ipool,
            tc.tile_pool(name="out", bufs=4) as opool,
            tc.tile_pool(name="psum", bufs=8, space="PSUM") as ppool,
        ):
            cf = cpool.tile([P, 2 * P], MYBIR_DT)
            nc.sync.dma_start(out=cf[:], in_=coef[:])
            cA = cf[:, 0:P]
            cB = cf[:, P : 2 * P]

            LAST = N_BLK - 1
            slabs = []
            for i in range(N_BLK):
                s = ipool.tile([P, ROWS], MYBIR_DT)
                if i == LAST:
                    # Final block: half-granular input and quarter-granular
                    # output so its writes are ready as the read stream ends
                    # (shortens the exposed tail chain).
                    h = ROWS // 2
                    nc.sync.dma_start(out=s[:, 0:h], in_=x_r[i][:, 0:h])
                    nc.sync.dma_start(out=s[:, h:ROWS], in_=x_r[i][:, h:ROWS])
                else:
                    nc.sync.dma_start(out=s[:], in_=x_r[i])
                slabs.append(s)

                utile = opool.tile([P, ROWS], MYBIR_DT)
                # All-A then all-B so the redundant-LDWEIGHTS dedup pass can
                # collapse each group to one weight load; the 8 chunks exactly
                # fill the 8 PSUM banks. Chunk direction alternates per block
                # so block i+1's A-matmuls only become ready (PSUM bank freed)
                # after block i's B-phase — keeping same-weight runs
                # contiguous in the scheduled PE order.
                order = list(range(N_CHUNK))
                if i % 2:
                    order.reverse()
                pts = {}
                for c in order:
                    pt = ppool.tile([P, CHUNK], mybir.dt.float32)
                    pts[c] = pt
                    sl = slice(c * CHUNK, (c + 1) * CHUNK)
                    if i > 0:
                        nc.tensor.matmul(
                            pt[:], lhsT=cA[:], rhs=slabs[i - 1][:, sl],
                            start=True, stop=False,
                        )
                copied = set()
                for c in order:
                    sl = slice(c * CHUNK, (c + 1) * CHUNK)
                    nc.tensor.matmul(
                        pts[c][:], lhsT=cB[:], rhs=slabs[i][:, sl],
                        start=(i == 0), stop=True,
                    )
                    if c % 2 == 0:
                        nc.vector.tensor_copy(utile[:, sl], pts[c][:])
                    else:
                        nc.scalar.copy(utile[:, sl], pts[c][:])
                    copied.add(c)
                    if i == LAST and (c ^ 1) in copied:
                        # final block streams output per chunk-pair so its
                        # writes are ready as the read stream ends
                        base = min(c, c ^ 1)
                        qs = slice(base * CHUNK, (base + 2) * CHUNK)
                        nc.scalar.dma_start(out=y_r[i][:, qs], in_=utile[:, qs])
                if i != LAST:
                    nc.scalar.dma_start(out=y_r[i], in_=utile[:])
                if i >= 1:
                    slabs[i - 1] = None

    _dedup_ldweights(nc)
    _split_excess_waits(nc)
    return nc


def kernel(x: np.ndarray, **_unused) -> np.ndarray:
    global _nc_cache, _coef_cache, last_results
    if _nc_cache is None:
        _nc_cache = _build()
        _coef_cache = _coef()
    nc = _nc_cache

    x = np.asarray(x)
    assert x.shape == (B, F, T), x.shape
    x16 = np.ascontiguousarray(x.reshape(N_CORES, ROWS, T), dtype=NP_DT)
    in_maps = [
        {"xt": np.ascontiguousarray(x16[c].T), "coef": _coef_cache}
        for c in range(N_CORES)
    ]
    last_results = run_bass_kernel_spmd(
        nc, in_maps, core_ids=list(range(N_CORES))
    )
    out = np.concatenate(
        [
            r["yt"].T.astype(np.float32).reshape(B_PER_CORE, F, T)
            for r in last_results.results
        ],
        axis=0,
    )
    return out
